# revision 6
# baseline (speedup 1.0000x reference)
"""AttnBlock (GroupNorm -> QKV 1x1 conv -> single-head attention over 4096
tokens -> proj -> residual) on 8 Trainium2 NeuronCores, batch-parallel
(one sample per core).

v3: fp8 transposes with engine-balanced phase B, GN affine folded into fp8
QKV weights, bf16 chain matmuls, act-table preloading (sqrt set early, exp
table loaded by a dummy during the fold), QKV fully emitted before the
attention loop, two-ahead score prefetch for a gapless exp stream, proj
matmuls spread across loop iterations, fast reciprocal for softmax sums.

Self-contained: hardcodes shapes b,h,w,c = 8,64,64,256 and builds/executes a
Bass/Tile kernel via run_bass_kernel_spmd.
"""

import sys

import numpy as np

if "/opt/trn_rl_repo" not in sys.path:
    sys.path.insert(0, "/opt/trn_rl_repo")

import concourse.bass as bass
import concourse.tile as tile
from concourse import bacc, mybir
from concourse.bass_utils import run_bass_kernel_spmd

F32 = mybir.dt.float32
BF16 = mybir.dt.bfloat16
FP8E4 = mybir.dt.float8e4  # e4m3 (TRN range +-240)
FP8E5 = mybir.dt.float8e5  # e5m2

B = 8
H = 64
W = 64
T = H * W          # 4096 tokens per sample
C = 256            # channels
P = 128            # partitions
CH = C // P        # 2 channel halves
TT = T // P        # 32 token tiles
QCS = 512          # q-chunk size (PSUM bank = 512 f32)
NQ = T // QCS      # 8 chunks
NS = T // (2 * P)  # 16 double-k-tile steps per q chunk
G = 32             # groups
GS = C // G        # 8 channels per group
EPS = 1e-6
N_GROUP = T * GS   # elements per group stat
SCALE = C ** -0.5  # softmax scale 1/16

AF = mybir.ActivationFunctionType
ALU = mybir.AluOpType
DR = mybir.MatmulPerfMode.DoubleRow


def _group_consts():
    gsel = np.zeros((P, CH, G), np.float32)   # [p, h, g] one-hot: channel->group
    gbro = np.zeros((G, CH, P), np.float32)   # [g, h, p] one-hot: group->channel
    for h in range(CH):
        for p in range(P):
            g = (h * P + p) // GS
            gsel[p, h, g] = 1.0
            gbro[g, h, p] = 1.0
    return gsel, gbro


def _emit(tc, nc, xd, wd, bd, gsd, gbd, inls, outd):
    ctxpools = []

    def pool(name, bufs, space="SBUF"):
        p = tc.alloc_tile_pool(name=name, bufs=bufs, space=space)
        ctxpools.append(p)
        return p

    const = pool("const", 1)
    stat = pool("stat", 1)
    work = pool("work", 2)
    epool = pool("epool", 8)
    # PSUM budget (8 banks): av0/av1/sps = 3, scores ring 2x2 = 4, small = 1
    ps_acc = pool("ps_acc", 1, space="PSUM")
    ps_sc = pool("ps_sc", 2, space="PSUM")
    ps_sm = pool("ps_sm", 1, space="PSUM")

    x_view = xd[:, :].rearrange("(n p) c -> p n c", p=P)
    out_view = outd[:, :].rearrange("(n p) c -> p n c", p=P)

    # ---------------- phase A: DMAs + consts ----------------
    big = pool("big", 1)
    x_nat = big.tile([P, TT, C], F32)     # natural layout, 4 MB (pre-biased +bp)
    ident_bf = const.tile([P, P], BF16)
    nc.sync.dma_start(out=ident_bf, in_=inls["ident_bf"][:, :])
    for i in range(8):
        nc.sync.dma_start(
            out=x_nat[:, i * 4:(i + 1) * 4, :], in_=x_view[:, i * 4:(i + 1) * 4, :]
        )

    gsel_sb = const.tile([P, CH, G], BF16)
    nc.sync.dma_start(out=gsel_sb, in_=inls["gsel"][:, :, :])
    gbro_sb = const.tile([G, CH, P], BF16)
    nc.sync.dma_start(out=gbro_sb, in_=inls["gbro"][:, :, :])
    onescol = const.tile([1, P], BF16)
    nc.sync.dma_start(out=onescol, in_=inls["onescol"][:, :])

    ones_e5 = const.tile([P, 2, P], FP8E5)
    nc.vector.memset(ones_e5, 1.0)

    # preload the sqrt_and_others act table (Copy/Square/Sqrt all live there)
    eps_sb = stat.tile([G, 1], F32)
    nc.vector.memset(eps_sb, EPS)
    dummy = stat.tile([G, 1], F32)
    nc.scalar.activation(out=dummy, in_=eps_sb, func=AF.Sqrt)

    wraw, wbb = {}, {}
    for nm in ("q", "k", "v", "p"):
        w_sb = const.tile([P, CH, C], F32, name=f"wraw_{nm}")
        nc.sync.dma_start(out=w_sb, in_=wd[nm][:, :].rearrange("(h p) d -> p h d", p=P))
        wraw[nm] = w_sb
    for nm in ("q", "k", "v"):
        wb = const.tile([P, CH, C], BF16, name=f"wbb_{nm}")
        nc.vector.tensor_copy(out=wb, in_=wraw[nm])
        wbb[nm] = wb
    wf = {"p": const.tile([P, CH, C], FP8E4, name="wf_p")}
    nc.vector.tensor_copy(out=wf["p"], in_=wraw["p"])

    bias_d = {}
    for nm in ("q", "k"):
        b_sb = const.tile([P, CH], F32, name=f"bias_{nm}")
        nc.sync.dma_start(out=b_sb, in_=bd[nm][:].rearrange("(h p) -> p h", p=P))
        bias_d[nm] = b_sb
    brow, brow_bf = {}, {}
    for nm in ("v", "p"):
        r_sb = const.tile([1, C], F32, name=f"brow_{nm}")
        nc.sync.dma_start(out=r_sb, in_=bd[nm][:].rearrange("(a c) -> a c", a=1))
        brow[nm] = r_sb
    brow_bf["p"] = const.tile([1, C], BF16, name="brow_bf_p")
    nc.vector.tensor_copy(out=brow_bf["p"], in_=brow["p"])

    gns_sb = const.tile([P, CH], F32)
    nc.sync.dma_start(out=gns_sb, in_=gsd[:].rearrange("(h p) -> p h", p=P))
    gnb_sb = const.tile([P, CH], F32)
    nc.sync.dma_start(out=gnb_sb, in_=gbd[:].rearrange("(h p) -> p h", p=P))

    # bp broadcast to [P, C] via PE (ones-column outer product)
    bp_ps = ps_sm.tile([P, C], F32, tag="small", name="bp_ps")
    nc.tensor.matmul(bp_ps, lhsT=onescol, rhs=brow_bf["p"], start=True, stop=True)
    bp_rep = const.tile([P, C], F32)
    nc.vector.tensor_copy(out=bp_rep, in_=bp_ps)

    # ---------------- persistent big tensors ----------------
    x_bf = big.tile([P, TT, C], BF16)     # bf16 copy of x for cheap transposes
    xT = big.tile([P, CH, T], FP8E4)      # x^T fp8
    qT = big.tile([P, CH, T], FP8E4)
    kT = big.tile([P, CH, T], FP8E4)
    v_sb = big.tile([P, TT, C], FP8E4)

    # ---------------- phase B: cast, transpose, stats, residual bias -------
    # st32 layout: [quarter, h, kind] -> col q*4 + h*2 + kind
    st32 = stat.tile([P, 16], F32)

    def stats_quarter(qq):
        sl = slice(qq * (T // 4), (qq + 1) * (T // 4))
        for h in range(CH):
            base = qq * 4 + 2 * h
            nc.vector.reduce_sum(
                out=st32[:, base:base + 1], in_=xT[:, h, sl],
                axis=mybir.AxisListType.X,
            )
            # Square writes into kT as scratch (overwritten later by QKV)
            nc.scalar.activation(
                out=kT[:, h, sl], in_=xT[:, h, sl], func=AF.Square,
                accum_out=st32[:, base + 1:base + 2],
            )

    for i in range(8):
        nc.scalar.activation(
            out=x_bf[:, 4 * i:4 * i + 4, :], in_=x_nat[:, 4 * i:4 * i + 4, :],
            func=AF.Copy,
        )
        for n in range(4 * i, 4 * i + 4):
            tp2 = ps_sc.tile([P, 2, P], BF16, tag="sc", name="tp2")
            for h in range(CH):
                nc.tensor.transpose(
                    tp2[:, h, :], x_bf[:, n, h * P:(h + 1) * P], ident_bf
                )
            if n % 4 == 0:
                nc.scalar.copy(out=xT[:, :, n * P:(n + 1) * P], in_=tp2)
            else:
                nc.vector.tensor_copy(out=xT[:, :, n * P:(n + 1) * P], in_=tp2)
            # residual pre-bias on the otherwise idle gpsimd engine
            nc.gpsimd.tensor_add(
                out=x_nat[:, n, :], in0=x_nat[:, n, :], in1=bp_rep
            )
        if i % 2 == 1:
            stats_quarter(i // 2)

    # ---------------- group stats -> m, a ----------------
    st8 = stat.tile([P, 8], F32)
    nc.vector.tensor_add(out=st8, in0=st32[:, 0:8], in1=st32[:, 8:16])
    st4 = stat.tile([P, 4], F32)
    nc.vector.tensor_add(out=st4, in0=st8[:, 0:4], in1=st8[:, 4:8])
    st4_bf = stat.tile([P, 4], BF16)
    nc.vector.tensor_copy(out=st4_bf, in_=st4)

    gps = ps_sm.tile([G, 2], F32, tag="small", name="gps")
    for h in range(CH):
        nc.tensor.matmul(
            gps, lhsT=gsel_sb[:, h, :], rhs=st4_bf[:, 2 * h:2 * h + 2],
            start=(h == 0), stop=(h == 1),
        )

    gstat = stat.tile([G, 4], F32)
    nc.vector.tensor_scalar_mul(out=gstat[:, 0:2], in0=gps, scalar1=1.0 / N_GROUP)
    nc.vector.tensor_mul(out=gstat[:, 2:3], in0=gstat[:, 0:1], in1=gstat[:, 0:1])
    nc.vector.tensor_sub(out=gstat[:, 2:3], in0=gstat[:, 1:2], in1=gstat[:, 2:3])
    nc.scalar.activation(
        out=gstat[:, 2:3], in_=gstat[:, 2:3], func=AF.Sqrt, bias=eps_sb, scale=1.0
    )
    nc.vector.reciprocal(out=gstat[:, 2:3], in_=gstat[:, 2:3])
    gmr = stat.tile([G, 2], BF16)
    nc.vector.tensor_copy(out=gmr[:, 0:1], in_=gstat[:, 0:1])
    nc.vector.tensor_copy(out=gmr[:, 1:2], in_=gstat[:, 2:3])

    # preload the exp table while the fold chain runs on PE/DVE
    nc.scalar.activation(out=dummy, in_=eps_sb, func=AF.Exp)

    mr_sb = stat.tile([P, CH, 2], F32)  # per-channel [mean, rstd]
    for h in range(CH):
        mbc = ps_sm.tile([P, 2], F32, tag="small", name="mbc")
        nc.tensor.matmul(mbc, lhsT=gbro_sb[:, h, :], rhs=gmr, start=True, stop=True)
        nc.vector.tensor_copy(out=mr_sb[:, h, :], in_=mbc)

    m_sb = stat.tile([P, CH], F32)
    a_sb = stat.tile([P, CH], F32)
    for h in range(CH):
        nc.vector.tensor_mul(
            out=m_sb[:, h:h + 1], in0=mr_sb[:, h, 1:2], in1=gns_sb[:, h:h + 1]
        )
        nc.vector.tensor_mul(
            out=a_sb[:, h:h + 1], in0=mr_sb[:, h, 0:1], in1=m_sb[:, h:h + 1]
        )
        nc.vector.tensor_sub(
            out=a_sb[:, h:h + 1], in0=gnb_sb[:, h:h + 1], in1=a_sb[:, h:h + 1]
        )
    a_bf = stat.tile([P, CH], BF16)
    nc.vector.tensor_copy(out=a_bf, in_=a_sb)

    # ---------------- fold GN affine into weights ----------------
    for nm in ("q", "k", "v"):
        wtile = const.tile([P, CH, C], FP8E4, name=f"wf_{nm}")
        for h in range(CH):
            nc.vector.tensor_scalar_mul(
                out=wtile[:, h, :], in0=wbb[nm][:, h, :], scalar1=m_sb[:, h:h + 1]
            )
        wf[nm] = wtile

    bias_qk = {}
    for nm in ("q", "k"):
        bf = const.tile([P, CH], F32, name=f"biasf_{nm}")
        for dh in range(CH):
            bb = ps_sm.tile([P, 1], F32, tag="small", name="bb")
            for h in range(CH):
                nc.tensor.matmul(
                    bb, lhsT=wbb[nm][:, h, dh * P:(dh + 1) * P],
                    rhs=a_bf[:, h:h + 1], start=(h == 0), stop=(h == 1),
                    skip_group_check=True,
                )
            nc.vector.tensor_add(
                out=bf[:, dh:dh + 1], in0=bb, in1=bias_d[nm][:, dh:dh + 1]
            )
        bias_qk[nm] = bf

    vrow_ps = ps_sm.tile([1, C], F32, tag="small", name="vrow_ps")
    for h in range(CH):
        nc.tensor.matmul(
            vrow_ps, lhsT=a_bf[:, h:h + 1], rhs=wbb["v"][:, h, :],
            start=(h == 0), stop=(h == 1), skip_group_check=True,
        )
    vrow = const.tile([1, C], BF16)
    nc.vector.tensor_add(out=vrow, in0=vrow_ps, in1=brow["v"])
    bv_ps = ps_sm.tile([P, C], F32, tag="small", name="bv_ps")
    nc.tensor.matmul(bv_ps, lhsT=onescol, rhs=vrow, start=True, stop=True)
    bvtot = const.tile([P, C], F32)
    nc.vector.tensor_copy(out=bvtot, in_=bv_ps)

    # ---------------- phase C: QKV (all chunks, DR fp8) ----------------
    def emit_qkv(ck):
        sl = slice(ck * QCS, (ck + 1) * QCS)
        for nm, dst in (("k", kT), ("q", qT)):
            ps = ps_sc.tile([P, CH, QCS], F32, tag="sc", name="psqk")
            for dh in range(CH):
                nc.tensor.matmul(
                    ps[:, dh, :], lhsT=wf[nm][:, :, dh * P:(dh + 1) * P],
                    rhs=xT[:, :, sl], start=True, stop=True, perf_mode=DR,
                )
            for dh in range(CH):
                nc.vector.tensor_scalar_add(
                    out=dst[:, dh, sl], in0=ps[:, dh, :],
                    scalar1=bias_qk[nm][:, dh:dh + 1],
                )
        for pair in range(2):
            psv = ps_sc.tile([P, 2, C], F32, tag="sc", name="psv")
            for j in range(2):
                n = 4 * ck + 2 * pair + j
                nc.tensor.matmul(
                    psv[:, j, :], lhsT=xT[:, :, n * P:(n + 1) * P],
                    rhs=wf["v"][:, :, :], start=True, stop=True, perf_mode=DR,
                )
            for j in range(2):
                n = 4 * ck + 2 * pair + j
                nc.vector.tensor_add(
                    out=v_sb[:, n, :], in0=psv[:, j, :], in1=bvtot
                )

    for ck in range(NQ):
        emit_qkv(ck)

    # ---------------- phase D: attention ----------------
    sc_tiles = {}
    drain = {}          # qc -> (ao, o_sb) carried into the next q-chunk

    def emit_scores(qc, s):
        qsl = slice(qc * QCS, (qc + 1) * QCS)
        scp = ps_sc.tile([P, 2, QCS], F32, tag="sc", name="scp")
        for j in range(2):
            kt = 2 * s + j
            nc.tensor.matmul(
                scp[:, j, :], lhsT=kT[:, :, kt * P:(kt + 1) * P],
                rhs=qT[:, :, qsl], start=True, stop=True, perf_mode=DR,
            )
        sc_tiles[(qc, s)] = scp

    def emit_proj_mm(qc, tt):
        ao, o_sb = drain[qc]
        po = ps_sm.tile([P, C], F32, tag="small", name="po")
        nc.tensor.matmul(
            po, lhsT=ao[:, :, tt * P:(tt + 1) * P], rhs=wf["p"][:, :, :],
            start=True, stop=True, perf_mode=DR,
        )
        n = qc * 4 + tt
        nc.vector.tensor_add(out=o_sb[:, tt, :], in0=po, in1=x_nat[:, n, :])

    def emit_out(qc):
        _, o_sb = drain.pop(qc)
        nc.sync.dma_start(
            out=out_view[:, qc * 4:qc * 4 + 4, :], in_=o_sb
        )

    emit_scores(0, 0)
    emit_scores(0, 1)
    for qc in range(NQ):
        av0 = ps_acc.tile([P, QCS], F32, tag="av0")
        av1 = ps_acc.tile([P, QCS], F32, tag="av1")
        sps = ps_acc.tile([P, QCS], F32, tag="sps")
        for s in range(NS):
            scp = sc_tiles.pop((qc, s))
            e_pair = epool.tile([P, 2, QCS], FP8E5, tag="e", name="e_pair")
            nc.scalar.activation(out=e_pair, in_=scp, func=AF.Exp, scale=SCALE)
            # two-ahead prefetch: scores(s+2) lands in the psum slot freed
            # by exp(s), keeping the exp stream gapless
            t = s + 2
            if t < NS:
                emit_scores(qc, t)
            elif qc + 1 < NQ:
                emit_scores(qc + 1, t - NS)
            first = s == 0
            last = s == NS - 1
            nc.tensor.matmul(
                av0, lhsT=v_sb[:, 2 * s:2 * s + 2, 0:P], rhs=e_pair,
                start=first, stop=last, perf_mode=DR, skip_group_check=True,
            )
            nc.tensor.matmul(
                av1, lhsT=v_sb[:, 2 * s:2 * s + 2, P:C], rhs=e_pair,
                start=first, stop=last, perf_mode=DR, skip_group_check=True,
            )
            nc.tensor.matmul(
                sps, lhsT=ones_e5, rhs=e_pair,
                start=first, stop=last, perf_mode=DR, skip_group_check=True,
            )
            # spread the previous chunk's proj matmuls one per iteration
            if qc > 0 and s in (7, 9, 11, 13):
                emit_proj_mm(qc - 1, (s - 7) // 2)
            if qc > 0 and s == 15:
                emit_out(qc - 1)
        # free the accumulation banks quickly, then normalize off-path
        av0_sb = work.tile([P, QCS], F32, tag="av0c", name="av0_sb")
        av1_sb = work.tile([P, QCS], F32, tag="av1c", name="av1_sb")
        sps_sb = work.tile([P, QCS], F32, tag="spsc", name="sps_sb")
        nc.vector.tensor_copy(out=av0_sb, in_=av0)
        nc.vector.tensor_copy(out=av1_sb, in_=av1)
        nc.vector.tensor_copy(out=sps_sb, in_=sps)
        r = work.tile([P, QCS], F32, tag="r", name="r")
        nc.vector.reciprocal_approx_fast(out=r, in_=sps_sb)
        ao = work.tile([P, CH, QCS], FP8E4, tag="ao", name="ao")
        nc.vector.tensor_mul(out=ao[:, 0, :], in0=av0_sb, in1=r)
        nc.vector.tensor_mul(out=ao[:, 1, :], in0=av1_sb, in1=r)
        o_sb = work.tile([P, 4, C], F32, tag="o", name="o_sb")
        drain[qc] = (ao, o_sb)
    for tt in range(4):
        emit_proj_mm(NQ - 1, tt)
    emit_out(NQ - 1)

    for p in reversed(ctxpools):
        p.release()


def build_nc():
    nc = bacc.Bacc()
    xd = nc.dram_tensor("x", [T, C], F32, kind="ExternalInput")
    wd, bd = {}, {}
    for nm in ("q", "k", "v", "p"):
        wd[nm] = nc.dram_tensor(f"w{nm}", [C, C], F32, kind="ExternalInput")
        bd[nm] = nc.dram_tensor(f"b{nm}", [C], F32, kind="ExternalInput")
    gsd = nc.dram_tensor("gn_scale", [C], F32, kind="ExternalInput")
    gbd = nc.dram_tensor("gn_bias", [C], F32, kind="ExternalInput")
    outd = nc.dram_tensor("out", [T, C], F32, kind="ExternalOutput")

    import ml_dtypes

    gsel_np, gbro_np = _group_consts()
    inls = {
        "gsel": nc.inline_tensor(gsel_np.astype(ml_dtypes.bfloat16), "gsel"),
        "gbro": nc.inline_tensor(gbro_np.astype(ml_dtypes.bfloat16), "gbro"),
        "ident_bf": nc.inline_tensor(
            np.eye(P, dtype=np.float32).astype(ml_dtypes.bfloat16), "ident_bf"
        ),
        "onescol": nc.inline_tensor(
            np.ones((1, P), dtype=ml_dtypes.bfloat16), "onescol"
        ),
    }

    with tile.TileContext(nc) as tc:
        _emit(tc, nc, xd, wd, bd, gsd, gbd, inls, outd)
    nc.compile()
    return nc


_CACHE = {}


def kernel(**inputs):
    x = np.asarray(inputs["x"], np.float32)
    assert x.shape == (B, H, W, C), x.shape
    if "nc" not in _CACHE:
        _CACHE["nc"] = build_nc()
    nc = _CACHE["nc"]

    shared = {}
    for nm in ("q", "k", "v", "p"):
        shared[f"w{nm}"] = np.ascontiguousarray(np.asarray(inputs[f"w{nm}"], np.float32))
        shared[f"b{nm}"] = np.ascontiguousarray(np.asarray(inputs[f"b{nm}"], np.float32))
    shared["gn_scale"] = np.ascontiguousarray(np.asarray(inputs["gn_scale"], np.float32))
    shared["gn_bias"] = np.ascontiguousarray(np.asarray(inputs["gn_bias"], np.float32))

    in_maps = []
    for i in range(B):
        m = dict(shared)
        m["x"] = np.ascontiguousarray(x[i].reshape(T, C))
        in_maps.append(m)

    res = run_bass_kernel_spmd(nc, in_maps, core_ids=list(range(B)))
    _CACHE["last_exec_time_ns"] = res.exec_time_ns
    out = np.stack([res.results[i]["out"].reshape(H, W, C) for i in range(B)], axis=0)
    return out


# revision 9
# speedup vs baseline: 1.2807x; 1.2807x over previous
"""AttnBlock (GroupNorm -> QKV 1x1 conv -> single-head attention over 4096
tokens -> proj -> residual) on 8 Trainium2 NeuronCores, batch-parallel
(one sample per core).

v4: plain exp (softmax scale folded 0.25/0.25 into the q/k conversions),
v-bias folded through the softmax (av' = av + rowsum*bv), bf16 transposes
with 1-bank batched psum copies, halved stats passes, engine-balanced
conversions (bias applied free in ACT Identity / DVE tensor_scalar),
two-ahead score prefetch, spread proj matmuls, fast reciprocal.

Self-contained: hardcodes shapes b,h,w,c = 8,64,64,256 and builds/executes a
Bass/Tile kernel via run_bass_kernel_spmd.
"""

import sys

import numpy as np

if "/opt/trn_rl_repo" not in sys.path:
    sys.path.insert(0, "/opt/trn_rl_repo")

import concourse.bass as bass
import concourse.tile as tile
from concourse import bacc, mybir
from concourse.bass_utils import run_bass_kernel_spmd

F32 = mybir.dt.float32
BF16 = mybir.dt.bfloat16
FP8E4 = mybir.dt.float8e4  # e4m3 (TRN range +-240)
FP8E5 = mybir.dt.float8e5  # e5m2

B = 8
H = 64
W = 64
T = H * W          # 4096 tokens per sample
C = 256            # channels
P = 128            # partitions
CH = C // P        # 2 channel halves
TT = T // P        # 32 token tiles
QCS = 512          # q-chunk size (PSUM bank = 512 f32)
NQ = T // QCS      # 8 chunks
NS = T // (2 * P)  # 16 double-k-tile steps per q chunk
G = 32             # groups
GS = C // G        # 8 channels per group
EPS = 1e-6
N_GROUP = T * GS   # elements per group stat
QKS = 0.25         # per-side score scale: (q*0.25)@(k*0.25) = qk/16

AF = mybir.ActivationFunctionType
ALU = mybir.AluOpType
DR = mybir.MatmulPerfMode.DoubleRow


def _group_consts():
    gsel = np.zeros((P, CH, G), np.float32)   # [p, h, g] one-hot: channel->group
    gbro = np.zeros((G, CH, P), np.float32)   # [g, h, p] one-hot: group->channel
    for h in range(CH):
        for p in range(P):
            g = (h * P + p) // GS
            gsel[p, h, g] = 1.0
            gbro[g, h, p] = 1.0
    return gsel, gbro


def _emit(tc, nc, xd, wd, bd, gsd, gbd, inls, outd):
    ctxpools = []

    def pool(name, bufs, space="SBUF"):
        p = tc.alloc_tile_pool(name=name, bufs=bufs, space=space)
        ctxpools.append(p)
        return p

    const = pool("const", 1)
    stat = pool("stat", 1)
    work = pool("work", 2)
    epool = pool("epool", 8)
    # PSUM budget (8 banks): av0/av1/sps = 3, scores ring 2x2 = 4, small = 1
    ps_acc = pool("ps_acc", 1, space="PSUM")
    ps_sc = pool("ps_sc", 2, space="PSUM")
    ps_sm = pool("ps_sm", 1, space="PSUM")

    x_view = xd[:, :].rearrange("(n p) c -> p n c", p=P)
    out_view = outd[:, :].rearrange("(n p) c -> p n c", p=P)

    # ---------------- phase A: DMAs + consts ----------------
    big = pool("big", 1)
    x_nat = big.tile([P, TT, C], F32)     # natural layout, 4 MB
    ident_bf = const.tile([P, P], BF16)
    nc.sync.dma_start(out=ident_bf, in_=inls["ident_bf"][:, :])
    for i in range(8):
        nc.sync.dma_start(
            out=x_nat[:, i * 4:(i + 1) * 4, :], in_=x_view[:, i * 4:(i + 1) * 4, :]
        )

    gsel_sb = const.tile([P, CH, G], BF16)
    nc.sync.dma_start(out=gsel_sb, in_=inls["gsel"][:, :, :])
    gbro_sb = const.tile([G, CH, P], BF16)
    nc.sync.dma_start(out=gbro_sb, in_=inls["gbro"][:, :, :])
    onescol = const.tile([1, P], BF16)
    nc.sync.dma_start(out=onescol, in_=inls["onescol"][:, :])

    ones_e5 = const.tile([P, 2, P], FP8E5)
    nc.vector.memset(ones_e5, 1.0)

    # preload the sqrt_and_others act table (Copy/Identity/Square/Sqrt)
    eps_sb = stat.tile([G, 1], F32)
    nc.vector.memset(eps_sb, EPS)
    dummy = stat.tile([G, 1], F32)
    nc.scalar.activation(out=dummy, in_=eps_sb, func=AF.Sqrt)

    wraw, wbb = {}, {}
    for nm in ("q", "k", "v", "p"):
        w_sb = const.tile([P, CH, C], F32, name=f"wraw_{nm}")
        nc.sync.dma_start(out=w_sb, in_=wd[nm][:, :].rearrange("(h p) d -> p h d", p=P))
        wraw[nm] = w_sb
    for nm in ("q", "k", "v"):
        wb = const.tile([P, CH, C], BF16, name=f"wbb_{nm}")
        nc.vector.tensor_copy(out=wb, in_=wraw[nm])
        wbb[nm] = wb
    wf = {"p": const.tile([P, CH, C], FP8E4, name="wf_p")}
    nc.vector.tensor_copy(out=wf["p"], in_=wraw["p"])

    bias_d = {}
    for nm in ("q", "k", "v"):
        b_sb = const.tile([P, CH], F32, name=f"bias_{nm}")
        nc.sync.dma_start(out=b_sb, in_=bd[nm][:].rearrange("(h p) -> p h", p=P))
        bias_d[nm] = b_sb
    brow_p = const.tile([1, C], F32, name="brow_p")
    nc.sync.dma_start(out=brow_p, in_=bd["p"][:].rearrange("(a c) -> a c", a=1))
    brow_bf = const.tile([1, C], BF16, name="brow_bf_p")
    nc.vector.tensor_copy(out=brow_bf, in_=brow_p)

    gns_sb = const.tile([P, CH], F32)
    nc.sync.dma_start(out=gns_sb, in_=gsd[:].rearrange("(h p) -> p h", p=P))
    gnb_sb = const.tile([P, CH], F32)
    nc.sync.dma_start(out=gnb_sb, in_=gbd[:].rearrange("(h p) -> p h", p=P))

    # bp broadcast to [P, C] via PE (ones-column outer product)
    bp_ps = ps_sm.tile([P, C], F32, tag="small", name="bp_ps")
    nc.tensor.matmul(bp_ps, lhsT=onescol, rhs=brow_bf, start=True, stop=True)
    bp_rep = const.tile([P, C], F32)
    nc.vector.tensor_copy(out=bp_rep, in_=bp_ps)

    # ---------------- persistent big tensors ----------------
    x_bf = big.tile([P, TT, C], BF16)     # bf16 copy of x for cheap transposes
    xT = big.tile([P, CH, T], FP8E4)      # x^T fp8
    qT = big.tile([P, CH, T], FP8E4)
    kT = big.tile([P, CH, T], FP8E4)
    v_sb = big.tile([P, TT, C], FP8E4)

    # ---------------- phase B: cast, transpose, stats ----------------
    # st16 layout: [half, h, kind] -> col hf*4 + h*2 + kind
    st16 = stat.tile([P, 8], F32)

    def stats_half(hf):
        sl = slice(hf * (T // 2), (hf + 1) * (T // 2))
        for h in range(CH):
            base = hf * 4 + 2 * h
            nc.vector.reduce_sum(
                out=st16[:, base:base + 1], in_=xT[:, h, sl],
                axis=mybir.AxisListType.X,
            )
            # Square writes into kT as scratch (overwritten later by QKV)
            nc.scalar.activation(
                out=kT[:, h, sl], in_=xT[:, h, sl], func=AF.Square,
                accum_out=st16[:, base + 1:base + 2],
            )

    for pair in range(4):  # 2 chunks (8 tiles) per iteration
        csl = slice(8 * pair, 8 * pair + 8)
        if pair % 2 == 0:
            nc.scalar.activation(
                out=x_bf[:, csl, :], in_=x_nat[:, csl, :], func=AF.Copy
            )
        else:
            nc.vector.tensor_copy(out=x_bf[:, csl, :], in_=x_nat[:, csl, :])
        for nb in range(2 * pair, 2 * pair + 2):  # 4-tile transpose groups
            tp8 = ps_sc.tile([P, 4, CH, P], BF16, tag="sc", name="tp8")
            for j in range(4):
                n = 4 * nb + j
                for h in range(CH):
                    nc.tensor.transpose(
                        tp8[:, j, h, :], x_bf[:, n, h * P:(h + 1) * P], ident_bf
                    )
            src = tp8[:, :, :, :].rearrange("p j h t -> p h j t")
            dst = xT[:, :, nb * 512:(nb + 1) * 512].rearrange(
                "p h (j t) -> p h j t", j=4
            )
            if nb % 2 == 0:
                nc.scalar.copy(out=dst, in_=src)
            else:
                nc.vector.tensor_copy(out=dst, in_=src)
        if pair % 2 == 1:
            stats_half(pair // 2)

    # ---------------- group stats -> m, a ----------------
    st4 = stat.tile([P, 4], F32)
    nc.vector.tensor_add(out=st4, in0=st16[:, 0:4], in1=st16[:, 4:8])
    st4_bf = stat.tile([P, 4], BF16)
    nc.vector.tensor_copy(out=st4_bf, in_=st4)

    gps = ps_sm.tile([G, 2], F32, tag="small", name="gps")
    for h in range(CH):
        nc.tensor.matmul(
            gps, lhsT=gsel_sb[:, h, :], rhs=st4_bf[:, 2 * h:2 * h + 2],
            start=(h == 0), stop=(h == 1),
        )

    gstat = stat.tile([G, 4], F32)
    nc.vector.tensor_scalar_mul(out=gstat[:, 0:2], in0=gps, scalar1=1.0 / N_GROUP)
    nc.vector.tensor_mul(out=gstat[:, 2:3], in0=gstat[:, 0:1], in1=gstat[:, 0:1])
    nc.vector.tensor_sub(out=gstat[:, 2:3], in0=gstat[:, 1:2], in1=gstat[:, 2:3])
    nc.scalar.activation(
        out=gstat[:, 2:3], in_=gstat[:, 2:3], func=AF.Sqrt, bias=eps_sb, scale=1.0
    )
    nc.vector.reciprocal(out=gstat[:, 2:3], in_=gstat[:, 2:3])
    gmr = stat.tile([G, 2], BF16)
    nc.vector.tensor_copy(out=gmr[:, 0:1], in_=gstat[:, 0:1])
    nc.vector.tensor_copy(out=gmr[:, 1:2], in_=gstat[:, 2:3])

    # preload the exp table while the fold chain runs on PE/DVE
    nc.scalar.activation(out=dummy, in_=eps_sb, func=AF.Exp)

    mean_bc = stat.tile([P, CH], F32)
    rstd_bc = stat.tile([P, CH], F32)
    for h in range(CH):
        mbc = ps_sm.tile([P, 2], F32, tag="small", name="mbc")
        nc.tensor.matmul(mbc, lhsT=gbro_sb[:, h, :], rhs=gmr, start=True, stop=True)
        nc.vector.tensor_copy(out=mean_bc[:, h:h + 1], in_=mbc[:, 0:1])
        nc.vector.tensor_copy(out=rstd_bc[:, h:h + 1], in_=mbc[:, 1:2])

    # m = rstd*gamma ; a = beta - mean*m   (batched over both halves)
    m_sb = stat.tile([P, CH], F32)
    a_sb = stat.tile([P, CH], F32)
    nc.vector.tensor_mul(out=m_sb, in0=rstd_bc, in1=gns_sb)
    nc.vector.tensor_mul(out=a_sb, in0=mean_bc, in1=m_sb)
    nc.vector.tensor_sub(out=a_sb, in0=gnb_sb, in1=a_sb)
    a_bf = stat.tile([P, CH], BF16)
    nc.vector.tensor_copy(out=a_bf, in_=a_sb)

    # ---------------- fold GN affine into weights (ACT, scale-AP) --------
    for nm in ("q", "k", "v"):
        wtile = const.tile([P, CH, C], FP8E4, name=f"wf_{nm}")
        for h in range(CH):
            nc.scalar.activation(
                out=wtile[:, h, :], in_=wbb[nm][:, h, :], func=AF.Identity,
                scale=m_sb[:, h:h + 1],
            )
        wf[nm] = wtile

    # bias' = a^T W + b in [d-part, dh] column form; q/k pre-scaled by 0.25
    bias_col = {}
    for nm in ("q", "k", "v"):
        bf = const.tile([P, CH], F32, name=f"biasf_{nm}")
        for dh in range(CH):
            bb = ps_sm.tile([P, 1], F32, tag="small", name="bb")
            for h in range(CH):
                nc.tensor.matmul(
                    bb, lhsT=wbb[nm][:, h, dh * P:(dh + 1) * P],
                    rhs=a_bf[:, h:h + 1], start=(h == 0), stop=(h == 1),
                    skip_group_check=True,
                )
            nc.vector.tensor_add(
                out=bf[:, dh:dh + 1], in0=bb, in1=bias_d[nm][:, dh:dh + 1]
            )
        bias_col[nm] = bf
    # pre-scale q/k bias columns by 0.25 (matches the conversion scale)
    bq4 = const.tile([P, CH], F32)
    nc.vector.tensor_scalar_mul(out=bq4, in0=bias_col["q"], scalar1=QKS)
    bk4 = const.tile([P, CH], F32)
    nc.vector.tensor_scalar_mul(out=bk4, in0=bias_col["k"], scalar1=QKS)

    # ---------------- phase C: QKV (all chunks, DR fp8) ----------------
    def emit_qkv(ck):
        sl = slice(ck * QCS, (ck + 1) * QCS)
        for nm, dst in (("k", kT), ("q", qT)):
            ps = ps_sc.tile([P, CH, QCS], F32, tag="sc", name="psqk")
            for dh in range(CH):
                nc.tensor.matmul(
                    ps[:, dh, :], lhsT=wf[nm][:, :, dh * P:(dh + 1) * P],
                    rhs=xT[:, :, sl], start=True, stop=True, perf_mode=DR,
                )
            for dh in range(CH):
                if nm == "k":
                    nc.scalar.activation(
                        out=dst[:, dh, sl], in_=ps[:, dh, :], func=AF.Identity,
                        scale=QKS, bias=bk4[:, dh:dh + 1],
                    )
                else:
                    nc.vector.tensor_scalar(
                        out=dst[:, dh, sl], in0=ps[:, dh, :],
                        scalar1=QKS, scalar2=bq4[:, dh:dh + 1],
                        op0=ALU.mult, op1=ALU.add,
                    )
        for pair in range(2):
            psv = ps_sc.tile([P, 2, C], F32, tag="sc", name="psv")
            for j in range(2):
                n = 4 * ck + 2 * pair + j
                nc.tensor.matmul(
                    psv[:, j, :], lhsT=xT[:, :, n * P:(n + 1) * P],
                    rhs=wf["v"][:, :, :], start=True, stop=True, perf_mode=DR,
                )
            n0 = 4 * ck + 2 * pair
            if pair == 0:
                nc.scalar.copy(out=v_sb[:, n0:n0 + 2, :], in_=psv)
            else:
                nc.vector.tensor_copy(out=v_sb[:, n0:n0 + 2, :], in_=psv)

    for ck in range(NQ):
        emit_qkv(ck)

    # ---------------- phase D: attention ----------------
    sc_tiles = {}
    drain = {}          # qc -> (ao, o_sb)

    def emit_scores(qc, s):
        qsl = slice(qc * QCS, (qc + 1) * QCS)
        scp = ps_sc.tile([P, 2, QCS], F32, tag="sc", name="scp")
        for j in range(2):
            kt = 2 * s + j
            nc.tensor.matmul(
                scp[:, j, :], lhsT=kT[:, :, kt * P:(kt + 1) * P],
                rhs=qT[:, :, qsl], start=True, stop=True, perf_mode=DR,
            )
        sc_tiles[(qc, s)] = scp

    def emit_proj_mm(qc, tt):
        ao, o_sb = drain[qc]
        po = ps_sm.tile([P, C], F32, tag="small", name="po")
        nc.tensor.matmul(
            po, lhsT=ao[:, :, tt * P:(tt + 1) * P], rhs=wf["p"][:, :, :],
            start=True, stop=True, perf_mode=DR,
        )
        n = qc * 4 + tt
        nc.vector.tensor_add(out=o_sb[:, tt, :], in0=po, in1=x_nat[:, n, :])
        nc.vector.tensor_add(out=o_sb[:, tt, :], in0=o_sb[:, tt, :], in1=bp_rep)

    def emit_out(qc):
        _, o_sb = drain.pop(qc)
        nc.sync.dma_start(
            out=out_view[:, qc * 4:qc * 4 + 4, :], in_=o_sb
        )

    emit_scores(0, 0)
    emit_scores(0, 1)
    for qc in range(NQ):
        av0 = ps_acc.tile([P, QCS], F32, tag="av0")
        av1 = ps_acc.tile([P, QCS], F32, tag="av1")
        sps = ps_acc.tile([P, QCS], F32, tag="sps")
        for s in range(NS):
            scp = sc_tiles.pop((qc, s))
            e_pair = epool.tile([P, 2, QCS], FP8E5, tag="e", name="e_pair")
            nc.scalar.activation(out=e_pair, in_=scp, func=AF.Exp)
            # two-ahead prefetch: scores(s+2) lands in the psum slot freed
            # by exp(s), keeping the exp stream gapless
            t = s + 2
            if t < NS:
                emit_scores(qc, t)
            elif qc + 1 < NQ:
                emit_scores(qc + 1, t - NS)
            first = s == 0
            last = s == NS - 1
            nc.tensor.matmul(
                av0, lhsT=v_sb[:, 2 * s:2 * s + 2, 0:P], rhs=e_pair,
                start=first, stop=last, perf_mode=DR, skip_group_check=True,
            )
            nc.tensor.matmul(
                av1, lhsT=v_sb[:, 2 * s:2 * s + 2, P:C], rhs=e_pair,
                start=first, stop=last, perf_mode=DR, skip_group_check=True,
            )
            nc.tensor.matmul(
                sps, lhsT=ones_e5, rhs=e_pair,
                start=first, stop=last, perf_mode=DR, skip_group_check=True,
            )
            # spread the previous chunk's proj matmuls one per iteration
            if qc > 0 and s in (7, 9, 11, 13):
                emit_proj_mm(qc - 1, (s - 7) // 2)
            if qc > 0 and s == 15:
                emit_out(qc - 1)
        # free the accumulation banks quickly, then normalize off-path;
        # v-bias folds through the softmax: av' = av + rowsum*bv
        av0_sb = work.tile([P, QCS], F32, tag="av0c", name="av0_sb")
        av1_sb = work.tile([P, QCS], F32, tag="av1c", name="av1_sb")
        sps_sb = work.tile([P, QCS], F32, tag="spsc", name="sps_sb")
        nc.vector.tensor_copy(out=av0_sb, in_=av0)
        nc.vector.tensor_copy(out=av1_sb, in_=av1)
        nc.vector.tensor_copy(out=sps_sb, in_=sps)
        r = work.tile([P, QCS], F32, tag="r", name="r")
        nc.vector.reciprocal_approx_fast(out=r, in_=sps_sb)
        nc.vector.scalar_tensor_tensor(
            out=av0_sb, in0=sps_sb, scalar=bias_col["v"][:, 0:1], in1=av0_sb,
            op0=ALU.mult, op1=ALU.add,
        )
        nc.vector.scalar_tensor_tensor(
            out=av1_sb, in0=sps_sb, scalar=bias_col["v"][:, 1:2], in1=av1_sb,
            op0=ALU.mult, op1=ALU.add,
        )
        ao = work.tile([P, CH, QCS], FP8E4, tag="ao", name="ao")
        nc.vector.tensor_mul(out=ao[:, 0, :], in0=av0_sb, in1=r)
        nc.vector.tensor_mul(out=ao[:, 1, :], in0=av1_sb, in1=r)
        o_sb = work.tile([P, 4, C], F32, tag="o", name="o_sb")
        drain[qc] = (ao, o_sb)
    for tt in range(4):
        emit_proj_mm(NQ - 1, tt)
    emit_out(NQ - 1)

    for p in reversed(ctxpools):
        p.release()


def build_nc():
    nc = bacc.Bacc()
    xd = nc.dram_tensor("x", [T, C], F32, kind="ExternalInput")
    wd, bd = {}, {}
    for nm in ("q", "k", "v", "p"):
        wd[nm] = nc.dram_tensor(f"w{nm}", [C, C], F32, kind="ExternalInput")
        bd[nm] = nc.dram_tensor(f"b{nm}", [C], F32, kind="ExternalInput")
    gsd = nc.dram_tensor("gn_scale", [C], F32, kind="ExternalInput")
    gbd = nc.dram_tensor("gn_bias", [C], F32, kind="ExternalInput")
    outd = nc.dram_tensor("out", [T, C], F32, kind="ExternalOutput")

    import ml_dtypes

    gsel_np, gbro_np = _group_consts()
    inls = {
        "gsel": nc.inline_tensor(gsel_np.astype(ml_dtypes.bfloat16), "gsel"),
        "gbro": nc.inline_tensor(gbro_np.astype(ml_dtypes.bfloat16), "gbro"),
        "ident_bf": nc.inline_tensor(
            np.eye(P, dtype=np.float32).astype(ml_dtypes.bfloat16), "ident_bf"
        ),
        "onescol": nc.inline_tensor(
            np.ones((1, P), dtype=ml_dtypes.bfloat16), "onescol"
        ),
    }

    with tile.TileContext(nc) as tc:
        _emit(tc, nc, xd, wd, bd, gsd, gbd, inls, outd)
    nc.compile()
    return nc


_CACHE = {}


def kernel(**inputs):
    x = np.asarray(inputs["x"], np.float32)
    assert x.shape == (B, H, W, C), x.shape
    if "nc" not in _CACHE:
        _CACHE["nc"] = build_nc()
    nc = _CACHE["nc"]

    shared = {}
    for nm in ("q", "k", "v", "p"):
        shared[f"w{nm}"] = np.ascontiguousarray(np.asarray(inputs[f"w{nm}"], np.float32))
        shared[f"b{nm}"] = np.ascontiguousarray(np.asarray(inputs[f"b{nm}"], np.float32))
    shared["gn_scale"] = np.ascontiguousarray(np.asarray(inputs["gn_scale"], np.float32))
    shared["gn_bias"] = np.ascontiguousarray(np.asarray(inputs["gn_bias"], np.float32))

    in_maps = []
    for i in range(B):
        m = dict(shared)
        m["x"] = np.ascontiguousarray(x[i].reshape(T, C))
        in_maps.append(m)

    res = run_bass_kernel_spmd(nc, in_maps, core_ids=list(range(B)))
    _CACHE["last_exec_time_ns"] = res.exec_time_ns
    out = np.stack([res.results[i]["out"].reshape(H, W, C) for i in range(B)], axis=0)
    return out
